# revision 17
# baseline (speedup 1.0000x reference)
# CATS-SwiGLU decode kernel for TRN2 (8 NeuronCores, SPMD tensor-parallel).
#
# Reference computation (decode path, B=S=1):
#   x1    = silu(x @ Wgatet)                  [1,1,dff]
#   flags = |x1| > threshold
#   z     = where(flags, (x @ Wup.T) * x1, 0) [1,1,dff]
#   out   = z @ Wdownt                        [1,1,d]
#
# Sharding: d_ff (11008) split across 8 cores (1376 rows each). Each core
# computes its z slice and a full-width partial down-projection; the host
# sums the 8 partials (the all-reduce of the TP hint, done on host).
#
# The kernel is HBM-bound: every weight byte streams exactly once, so the
# wins come from shrinking bytes. Wgatet streams as fp16 (it decides the
# CATS flags, keep it accurate); Wup/Wdownt stream as int8 with per-row
# scales. The int8 tiles are host-interleaved so a DVE int16 view splits
# into two streams via shift ops (4x DVE perf mode: 2-byte dtypes), each
# written out as fp16 for the TensorEngine. Per-row scales s_u[f]*s_d[f]
# fold into the z vector (exact), so dequant itself is scale-free.
#
# All three GEMVs run on the TensorEngine as M=1 matmuls: x / z is the
# stationary operand (1-column LdWeights, ~1ns), weight tiles stream as
# the moving operand at ~1 column/cycle. Gate/up accumulate into PSUM rows
# [1,1376], transposed into [128,11] via K=1 matmuls against a ones scalar
# so z is partition-major, ready as the down-projection stationary.
# The threshold is baked into the mask op as an immediate (kernel cache is
# keyed on its value), and x arrives as [32,128] and is transposed on the
# DVE — both avoid slow 128-descriptor broadcast DMAs in the startup path.
import sys

for _p in ("/opt/trn_rl_repo",):
    if _p not in sys.path:
        sys.path.insert(0, _p)

import numpy as np

import concourse.bass as bass
import concourse.tile as tile
from concourse import bacc, mybir
from concourse.bass_utils import run_bass_kernel_spmd

D = 4096
FF = 11008
NCORES = 8
FSH = FF // NCORES            # 1376 rows of d_ff per core
NCH = (FSH + 127) // 128      # 11 f-chunks of <=128
LAST = FSH - 128 * (NCH - 1)  # 96 rows in the last chunk
NDC = D // 128                # 32 d-chunks
G = 4                         # d-chunks per gate/up DMA tile
NT = NDC // G                 # 8 DMA tiles per gate/up matrix
HD = D // 2                   # 2048: down-tile half width
F32 = mybir.dt.float32
F16 = mybir.dt.float16
I8 = mybir.dt.int8
I16 = mybir.dt.int16
ACT = mybir.ActivationFunctionType
AND = mybir.AluOpType.bitwise_and
LSHR = mybir.AluOpType.logical_shift_right

_CACHE = {}


def _build_nc(thr_value):
    nc = bacc.Bacc("TRN2", target_bir_lowering=False, debug=False)

    x_d = nc.dram_tensor("x", [NDC, 128], F16, kind="ExternalInput")
    wg_d = nc.dram_tensor("wg", [NT, 128, G * FSH], F16, kind="ExternalInput")
    wu_d = nc.dram_tensor("wu", [NT, 128, G * FSH], I8, kind="ExternalInput")
    wd_d = nc.dram_tensor("wd", [FSH, D], I8, kind="ExternalInput")
    sud_d = nc.dram_tensor("sud", [128, NCH], F32, kind="ExternalInput")
    out_d = nc.dram_tensor("out", [1, D], F32, kind="ExternalOutput")

    NSPL = ((0, 512), (512, 1024), (1024, FSH))

    with tile.TileContext(nc) as tc:
        with (
            tc.tile_pool(name="const", bufs=1) as const_pool,
            tc.tile_pool(name="wpool", bufs=5) as wpool,
            tc.tile_pool(name="u8pool", bufs=3) as u8pool,
            tc.tile_pool(name="u16pool", bufs=3) as u16pool,
            tc.tile_pool(name="d8pool", bufs=4) as d8pool,
            tc.tile_pool(name="d16pool", bufs=6) as d16pool,
            tc.tile_pool(name="acts", bufs=1) as acts,
        ):
            # x arrives [32,128]; transposing DMA (XBAR) lands it as
            # [128,32] chunk-major without a 128-descriptor broadcast storm
            x_sb = const_pool.tile([128, NDC], F16)
            nc.scalar.dma_start(out=x_sb[:], in_=x_d.ap(), transpose=True)
            one_sb = const_pool.tile([1, 1], F16)
            nc.vector.memset(one_sb[:], 1.0)

            # warm the silu_and_others ACT table while the DMA stream runs
            warm = acts.tile([1, 1], F32)
            nc.vector.memset(warm[:], 1.0)
            nc.scalar.activation(warm[:], warm[:], ACT.Silu)
            nc.scalar.activation(warm[:], warm[:], ACT.Abs)

            x1row_sb = acts.tile([1, FSH], F16)
            urow_sb = acts.tile([1, FSH], F16)
            x1s = acts.tile([128, NCH], F32)
            absx = acts.tile([128, NCH], F32)
            mask = acts.tile([128, NCH], F32)
            ztmp = acts.tile([128, NCH], F32)
            zmA = acts.tile([128, NCH], F32)
            zm_sb = acts.tile([128, NCH], F16)
            sud_sb = acts.tile([128, NCH], F32)
            out_sb = acts.tile([1, D], F32)

            with tc.tile_pool(name="psA", bufs=1, space="PSUM") as psA:
                x1row = psA.tile([1, FSH], F32)
                urow = psA.tile([1, FSH], F32)
                x1tr = psA.tile([128, NCH], F32)
                utr = psA.tile([128, NCH], F32)
                nc.vector.memset(x1tr[:], 0.0)
                nc.vector.memset(utr[:], 0.0)

                def mm(accrow, c, rhs_ap, n0, n1):
                    nc.tensor.matmul(
                        out=accrow[0:1, n0:n1],
                        lhsT=x_sb[:, c : c + 1],
                        rhs=rhs_ap,
                        start=(c == 0),
                        stop=(c == NDC - 1),
                    )

                def gate_tile(t):
                    wt = wpool.tile([128, G * FSH], F16, tag="w", name="wt")
                    nc.sync.dma_start(out=wt[:], in_=wg_d.ap()[t])
                    for g in range(G):
                        for n0, n1 in NSPL:
                            mm(x1row, G * t + g, wt[:, g * FSH + n0 : g * FSH + n1], n0, n1)

                def unpack_pair(v16, lo_t, hi_t, p=128):
                    # int16 byte-pair split. The DVE int ALU is 32-bit with a
                    # truncating int16 store: `and 0xFF` yields the unsigned
                    # low byte (host pre-biases it by +128; the f16 cast op
                    # un-biases), `lshr 8` auto-sign-extends the high byte.
                    # bitvec ops can't cast, so the i16->f16 casts are
                    # separate arithmetic ops writing in place (bitcast view).
                    nc.vector.tensor_scalar(
                        out=lo_t[:p, :], in0=v16, scalar1=255, scalar2=None, op0=AND
                    )
                    nc.vector.tensor_scalar(
                        out=hi_t[:p, :], in0=v16, scalar1=8, scalar2=None, op0=LSHR
                    )
                    nc.vector.tensor_scalar(
                        out=lo_t[:p, :].bitcast(F16), in0=lo_t[:p, :],
                        scalar1=-128.0, scalar2=None, op0=mybir.AluOpType.add,
                    )
                    nc.vector.tensor_scalar_mul(
                        hi_t[:p, :].bitcast(F16), hi_t[:p, :], 1.0
                    )

                def up_tile(t):
                    u8 = u8pool.tile([128, G * FSH], I8, tag="u8", name="u8")
                    nc.sync.dma_start(out=u8[:], in_=wu_d.ap()[t])
                    v16 = u8[:].bitcast(I16)  # [128, 2*FSH] int16 byte-pairs
                    uA = u16pool.tile([128, 2 * FSH], I16, tag="uA", name="uA")
                    uB = u16pool.tile([128, 2 * FSH], I16, tag="uB", name="uB")
                    unpack_pair(v16, uA, uB)
                    for g in range(G):
                        src = (uA if g < 2 else uB)[:].bitcast(F16)
                        base = (g % 2) * FSH
                        for n0, n1 in NSPL:
                            mm(urow, G * t + g, src[:, base + n0 : base + n1], n0, n1)

                def transpose_row(row_sb, dst):
                    # [1, FSH] row -> [128, NCH] partition-major via K=1 matmuls
                    for c in range(NCH):
                        pc = 128 if c < NCH - 1 else LAST
                        nc.tensor.matmul(
                            out=dst[:pc, c : c + 1],
                            lhsT=row_sb[0:1, c * 128 : c * 128 + pc],
                            rhs=one_sb[:],
                            start=True,
                            stop=True,
                        )

                for t in range(NT):
                    gate_tile(t)
                up_tile(0)
                up_tile(1)
                # x1 post-processing overlaps the up stream
                nc.scalar.copy(x1row_sb[:], x1row[:])
                # sud DMA posted here so the scalar ring stays clear at t=0
                nc.scalar.dma_start(out=sud_sb[:], in_=sud_d.ap())
                transpose_row(x1row_sb, x1tr)
                nc.scalar.activation(x1s[:], x1tr[:], ACT.Silu)
                nc.scalar.activation(absx[:], x1s[:], ACT.Abs)
                nc.vector.tensor_scalar(
                    out=mask[:],
                    in0=absx[:],
                    scalar1=float(thr_value),
                    scalar2=None,
                    op0=mybir.AluOpType.is_gt,
                )
                for t in range(2, NT):
                    up_tile(t)
                # split the PSUM->SBUF drain across Act+DVE: it gates the
                # down-projection start, so halve its latency
                HF = (FSH // 2) // 512 * 512  # 512-aligned split point
                nc.scalar.copy(urow_sb[0:1, :HF], urow[0:1, :HF])
                nc.vector.tensor_copy(urow_sb[0:1, HF:], urow[0:1, HF:])
                transpose_row(urow_sb, utr)
                nc.vector.tensor_mul(ztmp[:], utr[:], x1s[:])
                nc.vector.tensor_mul(zmA[:], ztmp[:], mask[:])
                nc.vector.tensor_mul(zm_sb[:], zmA[:], sud_sb[:])

            with tc.tile_pool(name="psB", bufs=1, space="PSUM") as psB:
                dn = psB.tile([1, D], F32)
                for c in range(NCH):
                    pc = 128 if c < NCH - 1 else LAST
                    d8 = d8pool.tile([128, D], I8, tag="d8", name="d8")
                    nc.sync.dma_start(
                        out=d8[:pc, :], in_=wd_d.ap()[c * 128 : c * 128 + pc, :]
                    )
                    v16 = d8[:pc, :].bitcast(I16)  # [pc, HD] int16 byte-pairs
                    dlo = d16pool.tile([128, HD], I16, tag="dlo", name="dlo")
                    dhi = d16pool.tile([128, HD], I16, tag="dhi", name="dhi")
                    unpack_pair(v16, dlo, dhi, p=pc)
                    for b in range(8):
                        src = (dlo if b < 4 else dhi)[:pc, :].bitcast(F16)
                        n0 = (b % 4) * 512
                        nc.tensor.matmul(
                            out=dn[0:1, b * 512 : (b + 1) * 512],
                            lhsT=zm_sb[:pc, c : c + 1],
                            rhs=src[:, n0 : n0 + 512],
                            start=(c == 0),
                            stop=(c == NCH - 1),
                        )
                # drain PSUM per bank as each accumulation closes; alternate
                # Act/DVE so the tail is half as long
                for b in range(8):
                    sl = slice(b * 512, (b + 1) * 512)
                    if b % 2 == 0:
                        nc.scalar.copy(out_sb[0:1, sl], dn[0:1, sl])
                    else:
                        nc.vector.tensor_copy(out_sb[0:1, sl], dn[0:1, sl])

            # two half stores so the first can fire while the tail drains
            nc.sync.dma_start(out=out_d.ap()[0:1, :HD], in_=out_sb[0:1, :HD])
            nc.sync.dma_start(out=out_d.ap()[0:1, HD:], in_=out_sb[0:1, HD:])

    nc.compile()
    return nc


def _get_nc(thr_value):
    key = ("nc", float(thr_value))
    if key not in _CACHE:
        _CACHE[key] = _build_nc(float(thr_value))
    return _CACHE[key]


def _quant_rows(w):
    """Per-row symmetric int8: returns (q int8, scale f32[rows])."""
    s = np.abs(w).max(axis=1) / 127.0
    s[s == 0] = 1.0
    q = np.clip(np.rint(w / s[:, None]), -127, 127).astype(np.int8)
    return q, s.astype(np.float32)


def _interleave_cols(a, b):
    """Byte-interleave two equal-shape int8 blocks along the last axis.
    Low byte is biased +128 (see unpack_pair), high byte raw."""
    out = np.empty(a.shape[:-1] + (2 * a.shape[-1],), dtype=np.int8)
    out[..., 0::2] = (a.astype(np.int16) + 128).astype(np.uint8).view(np.int8)
    out[..., 1::2] = b
    return out


def make_in_maps(x, Wup, Wgatet, Wdownt):
    """Shard full inputs into the 8 per-core input maps."""
    x16 = np.asarray(x, dtype=np.float32).reshape(D).astype(np.float16)
    xb = np.ascontiguousarray(x16.reshape(NDC, 128))       # [32, 128]
    Wg16 = np.asarray(Wgatet, dtype=np.float32).astype(np.float16)  # [D, FF]
    Wup = np.asarray(Wup, dtype=np.float32)                # [FF, D]
    Wdownt = np.asarray(Wdownt, dtype=np.float32)          # [FF, D]
    in_maps = []
    for i in range(NCORES):
        sl = slice(i * FSH, (i + 1) * FSH)
        wg = (
            Wg16[:, sl]
            .reshape(NT, G, 128, FSH)
            .transpose(0, 2, 1, 3)
            .reshape(NT, 128, G * FSH)
        )
        wg = np.ascontiguousarray(wg)                      # [NT, 128, G*FSH]

        qu, su = _quant_rows(Wup[sl, :])                   # [FSH, D], [FSH]
        # -> [NT, 128, (g, f)] layout, then byte-pair gA with gB halves
        ut = (
            qu.T.reshape(NT, G, 128, FSH)
            .transpose(0, 2, 1, 3)
            .reshape(NT, 128, G * FSH)
        )
        wu = _interleave_cols(ut[:, :, : 2 * FSH], ut[:, :, 2 * FSH :])

        qd, sd = _quant_rows(Wdownt[sl, :])                # [FSH, D], [FSH]
        wd = _interleave_cols(qd[:, :HD], qd[:, HD:])      # [FSH, D]

        sud = np.zeros(NCH * 128, dtype=np.float32)
        sud[:FSH] = su * sd
        sud = np.ascontiguousarray(sud.reshape(NCH, 128).T)  # [128, NCH]

        in_maps.append({"x": xb, "wg": wg, "wu": wu, "wd": wd, "sud": sud})
    return in_maps


def run_sharded(x, Wup, Wgatet, Wdownt, threshold, trace=False, tmpdir=None):
    """Run on the 8 NeuronCores; returns (full_output, BassKernelResults)."""
    thr = float(np.asarray(threshold, dtype=np.float32).reshape(()))
    nc = _get_nc(thr)
    in_maps = make_in_maps(x, Wup, Wgatet, Wdownt)
    res = run_bass_kernel_spmd(
        nc, in_maps, list(range(NCORES)), trace=trace, tmpdir=tmpdir
    )
    # un-shard: sum the 8 partial down-projections
    acc = np.zeros(D, dtype=np.float64)
    for r in res.results:
        acc += r["out"].reshape(D).astype(np.float64)
    out = acc.astype(np.float32).reshape(1, 1, D)
    return out, res


def kernel(x, Wup, Wgatet, Wdownt, threshold):
    out, _ = run_sharded(x, Wup, Wgatet, Wdownt, threshold)
    return out


# revision 18
# speedup vs baseline: 1.0201x; 1.0201x over previous
# CATS-SwiGLU decode kernel for TRN2 (8 NeuronCores, SPMD tensor-parallel).
#
# Reference computation (decode path, B=S=1):
#   x1    = silu(x @ Wgatet)                  [1,1,dff]
#   flags = |x1| > threshold
#   z     = where(flags, (x @ Wup.T) * x1, 0) [1,1,dff]
#   out   = z @ Wdownt                        [1,1,d]
#
# Sharding: d_ff (11008) split across 8 cores (1376 rows each). Each core
# computes its z slice and a full-width partial down-projection; the host
# sums the 8 partials (the all-reduce of the TP hint, done on host).
#
# The kernel streams every weight byte exactly once, so the wins come from
# shrinking bytes and keeping every engine busy:
#  - Wgatet streams as fp16 (it decides the CATS flags, keep it accurate);
#    Wup/Wdownt stream as int8 with per-row scales folded into the z vector
#    (exact, since z_f multiplies whole rows).
#  - int8 tiles are dequantized to fp16 by a single cast op, split between
#    the DVE (tensor_scalar, 2x perf mode) and the otherwise-idle Act
#    engine (Copy), chosen per tile so neither paces the pipeline.
#  - gate (fp16, no dequant) and up (int8, dequant) tiles are interleaved
#    in the stream so the DMA engines and the dequant engines stay busy
#    simultaneously instead of phase-by-phase.
#  - All GEMVs run on the TensorEngine as M=1 matmuls: x / z is the
#    stationary operand (1-column LdWeights, ~1ns), weight tiles stream as
#    the moving operand at ~1 column/cycle. Gate/up accumulate into PSUM
#    rows [1,1376], transposed to [128,11] via K=1 matmuls against a ones
#    scalar so z is partition-major, ready as the down stationary.
#  - The threshold is baked into the mask op as an immediate (kernel cache
#    keyed on its value); x arrives [32,128] via a transposing (XBAR) DMA.
import sys

for _p in ("/opt/trn_rl_repo",):
    if _p not in sys.path:
        sys.path.insert(0, _p)

import numpy as np

import concourse.bass as bass
import concourse.tile as tile
from concourse import bacc, mybir
from concourse.bass_utils import run_bass_kernel_spmd

D = 4096
FF = 11008
NCORES = 8
FSH = FF // NCORES            # 1376 rows of d_ff per core
NCH = (FSH + 127) // 128      # 11 f-chunks of <=128
LAST = FSH - 128 * (NCH - 1)  # 96 rows in the last chunk
NDC = D // 128                # 32 d-chunks
G = 4                         # d-chunks per gate/up DMA tile
NT = NDC // G                 # 8 DMA tiles per gate/up matrix
HD = D // 2                   # 2048: half output width
F32 = mybir.dt.float32
F16 = mybir.dt.float16
I8 = mybir.dt.int8
ACT = mybir.ActivationFunctionType

# which tiles dequantize on the Act engine (rest on DVE)
ACT_UP = {1, 3, 5}
ACT_DN = {1, 4, 7, 9}
# interleaved gate/up stream: gate finishes ~80% through so the x1 path
# overlaps the up tail; u6/u7 follow the x1 path
ORDER = [
    ("g", 0), ("u", 0), ("g", 1), ("u", 1), ("g", 2), ("u", 2), ("g", 3),
    ("u", 3), ("g", 4), ("u", 4), ("g", 5), ("g", 6), ("u", 5), ("g", 7),
]

_CACHE = {}


def _build_nc(thr_value):
    nc = bacc.Bacc("TRN2", target_bir_lowering=False, debug=False)

    x_d = nc.dram_tensor("x", [NDC, 128], F16, kind="ExternalInput")
    wg_d = nc.dram_tensor("wg", [NT, 128, G * FSH], F16, kind="ExternalInput")
    wu_d = nc.dram_tensor("wu", [NT, 128, G * FSH], I8, kind="ExternalInput")
    wd_d = nc.dram_tensor("wd", [FSH, D], I8, kind="ExternalInput")
    sud_d = nc.dram_tensor("sud", [128, NCH], F32, kind="ExternalInput")
    out_d = nc.dram_tensor("out", [1, D], F32, kind="ExternalOutput")

    NSPL = ((0, 512), (512, 1024), (1024, FSH))

    with tile.TileContext(nc) as tc:
        with (
            tc.tile_pool(name="const", bufs=1) as const_pool,
            tc.tile_pool(name="wpool", bufs=4) as wpool,
            tc.tile_pool(name="u8pool", bufs=3) as u8pool,
            tc.tile_pool(name="u16pool", bufs=3) as u16pool,
            tc.tile_pool(name="d8pool", bufs=3) as d8pool,
            tc.tile_pool(name="d16pool", bufs=4) as d16pool,
            tc.tile_pool(name="acts", bufs=1) as acts,
        ):
            # x arrives [32,128]; transposing DMA (XBAR) lands it as
            # [128,32] chunk-major without a 128-descriptor broadcast storm
            x_sb = const_pool.tile([128, NDC], F16)
            nc.scalar.dma_start(out=x_sb[:], in_=x_d.ap(), transpose=True)
            one_sb = const_pool.tile([1, 1], F16)
            nc.vector.memset(one_sb[:], 1.0)

            # warm the silu_and_others ACT table while the DMA stream runs
            warm = acts.tile([1, 1], F32)
            nc.vector.memset(warm[:], 1.0)
            nc.scalar.activation(warm[:], warm[:], ACT.Silu)
            nc.scalar.activation(warm[:], warm[:], ACT.Abs)

            x1row_sb = acts.tile([1, FSH], F16)
            urow_sb = acts.tile([1, FSH], F16)
            x1s = acts.tile([128, NCH], F32)
            absx = acts.tile([128, NCH], F32)
            mask = acts.tile([128, NCH], F32)
            ztmp = acts.tile([128, NCH], F32)
            zmA = acts.tile([128, NCH], F32)
            zm_sb = acts.tile([128, NCH], F16)
            sud_sb = acts.tile([128, NCH], F32)
            out_sb = acts.tile([1, D], F32)

            def cast_tile(dst_ap, src_ap, on_act):
                if on_act:
                    nc.scalar.copy(dst_ap, src_ap)
                else:
                    nc.vector.tensor_scalar_mul(dst_ap, src_ap, 1.0)

            with tc.tile_pool(name="psA", bufs=1, space="PSUM") as psA:
                x1row = psA.tile([1, FSH], F32)
                urow = psA.tile([1, FSH], F32)
                x1tr = psA.tile([128, NCH], F32)
                utr = psA.tile([128, NCH], F32)
                nc.vector.memset(x1tr[:], 0.0)
                nc.vector.memset(utr[:], 0.0)

                def mm(accrow, c, rhs_ap, n0, n1):
                    nc.tensor.matmul(
                        out=accrow[0:1, n0:n1],
                        lhsT=x_sb[:, c : c + 1],
                        rhs=rhs_ap,
                        start=(c == 0),
                        stop=(c == NDC - 1),
                    )

                def gate_tile(t):
                    wt = wpool.tile([128, G * FSH], F16, tag="w", name="wt")
                    nc.sync.dma_start(out=wt[:], in_=wg_d.ap()[t])
                    for g in range(G):
                        for n0, n1 in NSPL:
                            mm(x1row, G * t + g, wt[:, g * FSH + n0 : g * FSH + n1], n0, n1)

                def up_tile(t):
                    u8 = u8pool.tile([128, G * FSH], I8, tag="u8", name="u8")
                    nc.sync.dma_start(out=u8[:], in_=wu_d.ap()[t])
                    uf = u16pool.tile([128, G * FSH], F16, tag="uf", name="uf")
                    cast_tile(uf[:], u8[:], t in ACT_UP)
                    for g in range(G):
                        for n0, n1 in NSPL:
                            mm(urow, G * t + g, uf[:, g * FSH + n0 : g * FSH + n1], n0, n1)

                def transpose_row(row_sb, dst):
                    # [1, FSH] row -> [128, NCH] partition-major via K=1 matmuls
                    for c in range(NCH):
                        pc = 128 if c < NCH - 1 else LAST
                        nc.tensor.matmul(
                            out=dst[:pc, c : c + 1],
                            lhsT=row_sb[0:1, c * 128 : c * 128 + pc],
                            rhs=one_sb[:],
                            start=True,
                            stop=True,
                        )

                for kind, t in ORDER:
                    (gate_tile if kind == "g" else up_tile)(t)
                # x1 post-processing overlaps the up tail
                nc.scalar.copy(x1row_sb[:], x1row[:])
                nc.scalar.dma_start(out=sud_sb[:], in_=sud_d.ap())
                transpose_row(x1row_sb, x1tr)
                nc.scalar.activation(x1s[:], x1tr[:], ACT.Silu)
                nc.scalar.activation(absx[:], x1s[:], ACT.Abs)
                nc.vector.tensor_scalar(
                    out=mask[:],
                    in0=absx[:],
                    scalar1=float(thr_value),
                    scalar2=None,
                    op0=mybir.AluOpType.is_gt,
                )
                up_tile(6)
                up_tile(7)

                # prefetch + dequant the first down tiles now so the DMA and
                # dequant engines don't idle while the z-chain resolves
                dpre = []
                for c in range(3):
                    d8 = d8pool.tile([128, D], I8, tag="d8", name="d8")
                    nc.sync.dma_start(out=d8[:], in_=wd_d.ap()[c * 128 : (c + 1) * 128, :])
                    df = d16pool.tile([128, D], F16, tag="df", name="df")
                    cast_tile(df[:], d8[:], c in ACT_DN)
                    dpre.append(df)

                # split the PSUM->SBUF drain across Act+DVE: it gates the
                # down-projection start, so halve its latency
                HF = (FSH // 2) // 512 * 512  # 512-aligned split point
                nc.scalar.copy(urow_sb[0:1, :HF], urow[0:1, :HF])
                nc.vector.tensor_copy(urow_sb[0:1, HF:], urow[0:1, HF:])
                transpose_row(urow_sb, utr)
                nc.vector.tensor_mul(ztmp[:], utr[:], x1s[:])
                nc.vector.tensor_mul(zmA[:], ztmp[:], mask[:])
                nc.vector.tensor_mul(zm_sb[:], zmA[:], sud_sb[:])

            with tc.tile_pool(name="psB", bufs=1, space="PSUM") as psB:
                dn = psB.tile([1, D], F32)

                def down_mms(c, df):
                    pc = 128 if c < NCH - 1 else LAST
                    for b in range(8):
                        nc.tensor.matmul(
                            out=dn[0:1, b * 512 : (b + 1) * 512],
                            lhsT=zm_sb[:pc, c : c + 1],
                            rhs=df[:pc, b * 512 : (b + 1) * 512],
                            start=(c == 0),
                            stop=(c == NCH - 1),
                        )

                for c, df in enumerate(dpre):
                    down_mms(c, df)
                for c in range(3, NCH):
                    pc = 128 if c < NCH - 1 else LAST
                    d8 = d8pool.tile([128, D], I8, tag="d8", name="d8")
                    nc.sync.dma_start(
                        out=d8[:pc, :], in_=wd_d.ap()[c * 128 : c * 128 + pc, :]
                    )
                    df = d16pool.tile([128, D], F16, tag="df", name="df")
                    cast_tile(df[:pc, :], d8[:pc, :], c in ACT_DN)
                    down_mms(c, df)
                # drain PSUM per bank as each accumulation closes; alternate
                # Act/DVE so the tail is half as long
                for b in range(8):
                    sl = slice(b * 512, (b + 1) * 512)
                    if b % 2 == 0:
                        nc.scalar.copy(out_sb[0:1, sl], dn[0:1, sl])
                    else:
                        nc.vector.tensor_copy(out_sb[0:1, sl], dn[0:1, sl])

            # two half stores so the first can fire while the tail drains
            nc.sync.dma_start(out=out_d.ap()[0:1, :HD], in_=out_sb[0:1, :HD])
            nc.sync.dma_start(out=out_d.ap()[0:1, HD:], in_=out_sb[0:1, HD:])

    nc.compile()
    return nc


def _get_nc(thr_value):
    key = ("nc", float(thr_value))
    if key not in _CACHE:
        _CACHE[key] = _build_nc(float(thr_value))
    return _CACHE[key]


def _quant_rows(w):
    """Per-row symmetric int8: returns (q int8, scale f32[rows])."""
    s = np.abs(w).max(axis=1) / 127.0
    s[s == 0] = 1.0
    q = np.clip(np.rint(w / s[:, None]), -127, 127).astype(np.int8)
    return q, s.astype(np.float32)


def make_in_maps(x, Wup, Wgatet, Wdownt):
    """Shard full inputs into the 8 per-core input maps."""
    x16 = np.asarray(x, dtype=np.float32).reshape(D).astype(np.float16)
    xb = np.ascontiguousarray(x16.reshape(NDC, 128))       # [32, 128]
    Wg16 = np.asarray(Wgatet, dtype=np.float32).astype(np.float16)  # [D, FF]
    Wup = np.asarray(Wup, dtype=np.float32)                # [FF, D]
    Wdownt = np.asarray(Wdownt, dtype=np.float32)          # [FF, D]
    in_maps = []
    for i in range(NCORES):
        sl = slice(i * FSH, (i + 1) * FSH)
        wg = (
            Wg16[:, sl]
            .reshape(NT, G, 128, FSH)
            .transpose(0, 2, 1, 3)
            .reshape(NT, 128, G * FSH)
        )
        wg = np.ascontiguousarray(wg)                      # [NT, 128, G*FSH]

        qu, su = _quant_rows(Wup[sl, :])                   # [FSH, D], [FSH]
        wu = (
            qu.T.reshape(NT, G, 128, FSH)
            .transpose(0, 2, 1, 3)
            .reshape(NT, 128, G * FSH)
        )
        wu = np.ascontiguousarray(wu)                      # [NT, 128, G*FSH]

        qd, sd = _quant_rows(Wdownt[sl, :])                # [FSH, D], [FSH]
        wd = np.ascontiguousarray(qd)                      # [FSH, D]

        sud = np.zeros(NCH * 128, dtype=np.float32)
        sud[:FSH] = su * sd
        sud = np.ascontiguousarray(sud.reshape(NCH, 128).T)  # [128, NCH]

        in_maps.append({"x": xb, "wg": wg, "wu": wu, "wd": wd, "sud": sud})
    return in_maps


def run_sharded(x, Wup, Wgatet, Wdownt, threshold, trace=False, tmpdir=None):
    """Run on the 8 NeuronCores; returns (full_output, BassKernelResults)."""
    thr = float(np.asarray(threshold, dtype=np.float32).reshape(()))
    nc = _get_nc(thr)
    in_maps = make_in_maps(x, Wup, Wgatet, Wdownt)
    res = run_bass_kernel_spmd(
        nc, in_maps, list(range(NCORES)), trace=trace, tmpdir=tmpdir
    )
    # un-shard: sum the 8 partial down-projections
    acc = np.zeros(D, dtype=np.float64)
    for r in res.results:
        acc += r["out"].reshape(D).astype(np.float64)
    out = acc.astype(np.float32).reshape(1, 1, D)
    return out, res


def kernel(x, Wup, Wgatet, Wdownt, threshold):
    out, _ = run_sharded(x, Wup, Wgatet, Wdownt, threshold)
    return out


# revision 19
# speedup vs baseline: 1.0909x; 1.0694x over previous
# CATS-SwiGLU decode kernel for TRN2 (8 NeuronCores, SPMD tensor-parallel).
#
# Reference computation (decode path, B=S=1):
#   x1    = silu(x @ Wgatet)                  [1,1,dff]
#   flags = |x1| > threshold
#   z     = where(flags, (x @ Wup.T) * x1, 0) [1,1,dff]
#   out   = z @ Wdownt                        [1,1,d]
#
# Sharding: d_ff (11008) split across 8 cores (1376 rows each). Each core
# computes its z slice and a full-width partial down-projection; the host
# sums the 8 partials (the all-reduce of the TP hint, done on host).
#
# The kernel streams every weight byte exactly once; the design goals are
# fewer bytes and no idle engines:
#  - Wgatet streams as fp16 (it decides the CATS flags, keep it accurate);
#    Wup/Wdownt stream as int8 with per-row scales folded into the z vector
#    (exact, since z_f scales whole rows of Wdownt / single f-columns).
#  - int8 tiles are dequantized to fp16 one 1376-column chunk at a time,
#    alternating between the DVE (tensor_scalar, 2x perf mode) and the
#    otherwise-idle Act engine (Copy) so neither paces the pipeline.
#  - All DMAs are uniform ~1.4-2MB tiles on the sync HWDGE ring (mixed
#    small tiles measurably drop the 16-engine stream rate); gate and up
#    tiles interleave so dequant overlaps the fp16 stream, and the first
#    gate tile is split into 4 chunk DMAs so the PE starts early.
#  - All GEMVs run on the TensorEngine as M=1 matmuls: x / z is the
#    stationary operand (1-column LdWeights), weight tiles stream as the
#    moving operand at ~1 column/cycle. Gate/up accumulate into PSUM rows
#    [1,1376], transposed to [128,11] via K=1 matmuls against a ones
#    scalar so z is partition-major, ready as the down stationary.
#  - The threshold is baked into the mask op as an immediate (kernel cache
#    keyed on its value); x arrives [32,128] via a transposing (XBAR) DMA.
import sys

for _p in ("/opt/trn_rl_repo",):
    if _p not in sys.path:
        sys.path.insert(0, _p)

import numpy as np

import concourse.bass as bass
import concourse.tile as tile
from concourse import bacc, mybir
from concourse.bass_utils import run_bass_kernel_spmd

D = 4096
FF = 11008
NCORES = 8
FSH = FF // NCORES            # 1376 rows of d_ff per core
NCH = (FSH + 127) // 128      # 11 f-chunks of <=128
LAST = FSH - 128 * (NCH - 1)  # 96 rows in the last chunk
NDC = D // 128                # 32 d-chunks
G = 4                         # d-chunks per gate DMA tile
NT = NDC // G                 # 8 gate tiles
GU = 8                        # d-chunks per up DMA tile (int8: same bytes)
NTU = NDC // GU               # 4 up tiles
ND2 = 5                       # paired down tiles (chunks 0..9)
HD = D // 2                   # 2048: half output width
F32 = mybir.dt.float32
F16 = mybir.dt.float16
I8 = mybir.dt.int8
ACT = mybir.ActivationFunctionType

# interleaved stream: gate finishes ~80% through so the x1 path overlaps
# the up tail; u3 follows the x1 path
ORDER = [
    ("g", 0), ("u", 0), ("g", 1), ("g", 2), ("u", 1), ("g", 3), ("g", 4),
    ("g", 5), ("u", 2), ("g", 6), ("g", 7),
]

_CACHE = {}


def _build_nc(thr_value):
    nc = bacc.Bacc("TRN2", target_bir_lowering=False, debug=False)

    x_d = nc.dram_tensor("x", [NDC, 128], F16, kind="ExternalInput")
    wg_d = nc.dram_tensor("wg", [NT, 128, G * FSH], F16, kind="ExternalInput")
    wu_d = nc.dram_tensor("wu", [NTU, 128, GU * FSH], I8, kind="ExternalInput")
    wd_d = nc.dram_tensor("wd", [ND2, 128, 2 * D], I8, kind="ExternalInput")
    wdL_d = nc.dram_tensor("wdL", [LAST, D], I8, kind="ExternalInput")
    sud_d = nc.dram_tensor("sud", [128, NCH], F32, kind="ExternalInput")
    out_d = nc.dram_tensor("out", [1, D], F32, kind="ExternalOutput")

    NSPL = ((0, 512), (512, 1024), (1024, FSH))

    with tile.TileContext(nc) as tc:
        with (
            tc.tile_pool(name="const", bufs=1) as const_pool,
            tc.tile_pool(name="wpool", bufs=3) as wpool,
            tc.tile_pool(name="u8pool", bufs=2) as u8pool,
            tc.tile_pool(name="u16pool", bufs=2) as u16pool,
            tc.tile_pool(name="d8pool", bufs=2) as d8pool,
            tc.tile_pool(name="d16pool", bufs=3) as d16pool,
            tc.tile_pool(name="acts", bufs=1) as acts,
        ):
            # x arrives [32,128]; transposing DMA (XBAR) lands it as
            # [128,32] chunk-major without a 128-descriptor broadcast storm
            x_sb = const_pool.tile([128, NDC], F16)
            nc.scalar.dma_start(out=x_sb[:], in_=x_d.ap(), transpose=True)
            one_sb = const_pool.tile([1, 1], F16)
            nc.vector.memset(one_sb[:], 1.0)

            # warm the silu_and_others ACT table while the DMA stream runs
            warm = acts.tile([1, 1], F32)
            nc.vector.memset(warm[:], 1.0)
            nc.scalar.activation(warm[:], warm[:], ACT.Silu)
            nc.scalar.activation(warm[:], warm[:], ACT.Abs)

            x1row_sb = acts.tile([1, FSH], F16)
            urow_sb = acts.tile([1, FSH], F16)
            x1s = acts.tile([128, NCH], F32)
            absx = acts.tile([128, NCH], F32)
            mask = acts.tile([128, NCH], F32)
            ztmp = acts.tile([128, NCH], F32)
            zmA = acts.tile([128, NCH], F32)
            zm_sb = acts.tile([128, NCH], F16)
            sud_sb = acts.tile([128, NCH], F32)
            out_sb = acts.tile([1, D], F32)

            def cast_chunk(dst_ap, src_ap, on_act):
                if on_act:
                    nc.scalar.copy(dst_ap, src_ap)
                else:
                    nc.vector.tensor_scalar_mul(dst_ap, src_ap, 1.0)

            with tc.tile_pool(name="psA", bufs=1, space="PSUM") as psA:
                x1row = psA.tile([1, FSH], F32)
                urow = psA.tile([1, FSH], F32)
                x1tr = psA.tile([128, NCH], F32)
                utr = psA.tile([128, NCH], F32)
                nc.vector.memset(x1tr[:], 0.0)
                nc.vector.memset(utr[:], 0.0)

                def mm(accrow, c, rhs_ap, n0, n1):
                    nc.tensor.matmul(
                        out=accrow[0:1, n0:n1],
                        lhsT=x_sb[:, c : c + 1],
                        rhs=rhs_ap,
                        start=(c == 0),
                        stop=(c == NDC - 1),
                    )

                def gate_tile(t):
                    wt = wpool.tile([128, G * FSH], F16, tag="w", name="wt")
                    if t == 0:
                        # 4 chunk DMAs: the PE can start on the first chunk
                        # ~3us before a whole-tile transfer would land
                        for g in range(G):
                            cs = slice(g * FSH, (g + 1) * FSH)
                            nc.sync.dma_start(out=wt[:, cs], in_=wg_d.ap()[0][:, cs])
                    else:
                        nc.sync.dma_start(out=wt[:], in_=wg_d.ap()[t])
                    for g in range(G):
                        for n0, n1 in NSPL:
                            mm(x1row, G * t + g, wt[:, g * FSH + n0 : g * FSH + n1], n0, n1)

                def up_tile(t):
                    u8 = u8pool.tile([128, GU * FSH], I8, tag="u8", name="u8")
                    nc.sync.dma_start(out=u8[:], in_=wu_d.ap()[t])
                    uf = u16pool.tile([128, GU * FSH], F16, tag="uf", name="uf")
                    for g in range(GU):
                        cs = slice(g * FSH, (g + 1) * FSH)
                        cast_chunk(uf[:, cs], u8[:, cs], g % 8 in (2, 5, 7))
                        for n0, n1 in NSPL:
                            mm(urow, GU * t + g, uf[:, g * FSH + n0 : g * FSH + n1], n0, n1)

                def transpose_row(row_sb, dst):
                    # [1, FSH] row -> [128, NCH] partition-major via K=1 matmuls
                    for c in range(NCH):
                        pc = 128 if c < NCH - 1 else LAST
                        nc.tensor.matmul(
                            out=dst[:pc, c : c + 1],
                            lhsT=row_sb[0:1, c * 128 : c * 128 + pc],
                            rhs=one_sb[:],
                            start=True,
                            stop=True,
                        )

                for kind, t in ORDER:
                    (gate_tile if kind == "g" else up_tile)(t)
                # x1 post-processing overlaps the up tail
                nc.scalar.copy(x1row_sb[:], x1row[:])
                nc.scalar.dma_start(out=sud_sb[:], in_=sud_d.ap())
                transpose_row(x1row_sb, x1tr)
                nc.scalar.activation(x1s[:], x1tr[:], ACT.Silu)
                nc.scalar.activation(absx[:], x1s[:], ACT.Abs)
                nc.vector.tensor_scalar(
                    out=mask[:],
                    in0=absx[:],
                    scalar1=float(thr_value),
                    scalar2=None,
                    op0=mybir.AluOpType.is_gt,
                )
                up_tile(3)

                # prefetch + dequant the first down tiles now so the DMA and
                # dequant engines don't idle while the z-chain resolves
                dpre = []
                for t in range(2):
                    d8 = d8pool.tile([128, 2 * D], I8, tag="d8", name="d8")
                    nc.sync.dma_start(out=d8[:], in_=wd_d.ap()[t])
                    df = d16pool.tile([128, 2 * D], F16, tag="df", name="df")
                    for h in range(2):
                        hs = slice(h * D, (h + 1) * D)
                        cast_chunk(df[:, hs], d8[:, hs], (2 * t + h) % 2 == 1)
                    dpre.append(df)

                # split the PSUM->SBUF drain across Act+DVE: it gates the
                # down-projection start, so halve its latency
                HF = (FSH // 2) // 512 * 512  # 512-aligned split point
                nc.scalar.copy(urow_sb[0:1, :HF], urow[0:1, :HF])
                nc.vector.tensor_copy(urow_sb[0:1, HF:], urow[0:1, HF:])
                transpose_row(urow_sb, utr)
                nc.vector.tensor_mul(ztmp[:], utr[:], x1s[:])
                nc.vector.tensor_mul(zmA[:], ztmp[:], mask[:])
                nc.vector.tensor_mul(zm_sb[:], zmA[:], sud_sb[:])

            with tc.tile_pool(name="psB", bufs=1, space="PSUM") as psB:
                dn = psB.tile([1, D], F32)

                def down_mms(c, df_ap):
                    # df_ap: [pc, D] fp16 view of chunk c's dequantized rows
                    pc = 128 if c < NCH - 1 else LAST
                    for b in range(8):
                        nc.tensor.matmul(
                            out=dn[0:1, b * 512 : (b + 1) * 512],
                            lhsT=zm_sb[:pc, c : c + 1],
                            rhs=df_ap[:, b * 512 : (b + 1) * 512],
                            start=(c == 0),
                            stop=(c == NCH - 1),
                        )

                for t, df in enumerate(dpre):
                    down_mms(2 * t, df[:, :D])
                    down_mms(2 * t + 1, df[:, D:])
                for t in range(2, ND2):
                    d8 = d8pool.tile([128, 2 * D], I8, tag="d8", name="d8")
                    nc.sync.dma_start(out=d8[:], in_=wd_d.ap()[t])
                    df = d16pool.tile([128, 2 * D], F16, tag="df", name="df")
                    for h in range(2):
                        hs = slice(h * D, (h + 1) * D)
                        cast_chunk(df[:, hs], d8[:, hs], (2 * t + h) % 2 == 1)
                        down_mms(2 * t + h, df[:, hs])
                # last 96-row chunk
                d8L = d8pool.tile([128, 2 * D], I8, tag="d8", name="d8")
                nc.sync.dma_start(out=d8L[:LAST, :D], in_=wdL_d.ap())
                dfL = d16pool.tile([128, 2 * D], F16, tag="df", name="df")
                cast_chunk(dfL[:LAST, :D], d8L[:LAST, :D], False)
                down_mms(NCH - 1, dfL[:LAST, :D])
                # drain PSUM per bank as each accumulation closes; alternate
                # Act/DVE so the tail is half as long
                for b in range(8):
                    sl = slice(b * 512, (b + 1) * 512)
                    if b % 2 == 0:
                        nc.scalar.copy(out_sb[0:1, sl], dn[0:1, sl])
                    else:
                        nc.vector.tensor_copy(out_sb[0:1, sl], dn[0:1, sl])

            # two half stores so the first can fire while the tail drains
            nc.sync.dma_start(out=out_d.ap()[0:1, :HD], in_=out_sb[0:1, :HD])
            nc.sync.dma_start(out=out_d.ap()[0:1, HD:], in_=out_sb[0:1, HD:])

    nc.compile()
    return nc


def _get_nc(thr_value):
    key = ("nc", float(thr_value))
    if key not in _CACHE:
        _CACHE[key] = _build_nc(float(thr_value))
    return _CACHE[key]


def _quant_rows(w):
    """Per-row symmetric int8: returns (q int8, scale f32[rows])."""
    s = np.abs(w).max(axis=1) / 127.0
    s[s == 0] = 1.0
    q = np.clip(np.rint(w / s[:, None]), -127, 127).astype(np.int8)
    return q, s.astype(np.float32)


def make_in_maps(x, Wup, Wgatet, Wdownt):
    """Shard full inputs into the 8 per-core input maps."""
    x16 = np.asarray(x, dtype=np.float32).reshape(D).astype(np.float16)
    xb = np.ascontiguousarray(x16.reshape(NDC, 128))       # [32, 128]
    Wg16 = np.asarray(Wgatet, dtype=np.float32).astype(np.float16)  # [D, FF]
    Wup = np.asarray(Wup, dtype=np.float32)                # [FF, D]
    Wdownt = np.asarray(Wdownt, dtype=np.float32)          # [FF, D]
    in_maps = []
    for i in range(NCORES):
        sl = slice(i * FSH, (i + 1) * FSH)
        wg = (
            Wg16[:, sl]
            .reshape(NT, G, 128, FSH)
            .transpose(0, 2, 1, 3)
            .reshape(NT, 128, G * FSH)
        )
        wg = np.ascontiguousarray(wg)                      # [NT, 128, G*FSH]

        qu, su = _quant_rows(Wup[sl, :])                   # [FSH, D], [FSH]
        wu = (
            qu.T.reshape(NTU, GU, 128, FSH)
            .transpose(0, 2, 1, 3)
            .reshape(NTU, 128, GU * FSH)
        )
        wu = np.ascontiguousarray(wu)                      # [NTU, 128, GU*FSH]

        qd, sd = _quant_rows(Wdownt[sl, :])                # [FSH, D], [FSH]
        # paired down tiles: chunk 2t and 2t+1 side by side in the free dim
        wd = np.ascontiguousarray(
            qd[: ND2 * 256, :].reshape(ND2, 2, 128, D)
            .transpose(0, 2, 1, 3)
            .reshape(ND2, 128, 2 * D)
        )
        wdL = np.ascontiguousarray(qd[ND2 * 256 :, :])     # [96, D]

        sud = np.zeros(NCH * 128, dtype=np.float32)
        sud[:FSH] = su * sd
        sud = np.ascontiguousarray(sud.reshape(NCH, 128).T)  # [128, NCH]

        in_maps.append(
            {"x": xb, "wg": wg, "wu": wu, "wd": wd, "wdL": wdL, "sud": sud}
        )
    return in_maps


def run_sharded(x, Wup, Wgatet, Wdownt, threshold, trace=False, tmpdir=None):
    """Run on the 8 NeuronCores; returns (full_output, BassKernelResults)."""
    thr = float(np.asarray(threshold, dtype=np.float32).reshape(()))
    nc = _get_nc(thr)
    in_maps = make_in_maps(x, Wup, Wgatet, Wdownt)
    res = run_bass_kernel_spmd(
        nc, in_maps, list(range(NCORES)), trace=trace, tmpdir=tmpdir
    )
    # un-shard: sum the 8 partial down-projections
    acc = np.zeros(D, dtype=np.float64)
    for r in res.results:
        acc += r["out"].reshape(D).astype(np.float64)
    out = acc.astype(np.float32).reshape(1, 1, D)
    return out, res


def kernel(x, Wup, Wgatet, Wdownt, threshold):
    out, _ = run_sharded(x, Wup, Wgatet, Wdownt, threshold)
    return out


# revision 24
# speedup vs baseline: 1.0999x; 1.0082x over previous
# CATS-SwiGLU decode kernel for TRN2 (8 NeuronCores, SPMD tensor-parallel).
#
# Reference computation (decode path, B=S=1):
#   x1    = silu(x @ Wgatet)                  [1,1,dff]
#   flags = |x1| > threshold
#   z     = where(flags, (x @ Wup.T) * x1, 0) [1,1,dff]
#   out   = z @ Wdownt                        [1,1,d]
#
# Sharding: d_ff (11008) split across 8 cores (1376 rows each). Each core
# computes its z slice and a full-width partial down-projection; the host
# sums the 8 partials (the all-reduce of the TP hint, done on host).
#
# The kernel streams every weight byte exactly once; the design goals are
# fewer bytes and no idle engines:
#  - Wgatet streams as fp16 (it decides the CATS flags, keep it accurate);
#    Wup/Wdownt stream as int8 with per-row scales folded into the z vector
#    (exact, since z_f scales whole rows of Wdownt / single f-columns).
#  - int8 tiles are dequantized to fp16 one 1376-column chunk at a time,
#    alternating between the DVE (tensor_scalar, 2x perf mode) and the
#    otherwise-idle Act engine (Copy) so neither paces the pipeline.
#  - All DMAs are uniform ~1.4-2MB tiles on the sync HWDGE ring (mixed
#    small tiles measurably drop the 16-engine stream rate); gate and up
#    tiles interleave so dequant overlaps the fp16 stream, and the first
#    gate tile is split into 4 chunk DMAs so the PE starts early.
#  - All GEMVs run on the TensorEngine as M=1 matmuls: x / z is the
#    stationary operand (1-column LdWeights), weight tiles stream as the
#    moving operand at ~1 column/cycle. Gate/up accumulate into PSUM rows
#    [1,1376], transposed to [128,11] via K=1 matmuls against a ones
#    scalar so z is partition-major, ready as the down stationary.
#  - The threshold is baked into the mask op as an immediate (kernel cache
#    keyed on its value); x arrives [32,128] via a transposing (XBAR) DMA.
import sys

for _p in ("/opt/trn_rl_repo",):
    if _p not in sys.path:
        sys.path.insert(0, _p)

import numpy as np

import concourse.bass as bass
import concourse.tile as tile
from concourse import bacc, mybir
from concourse.bass_utils import run_bass_kernel_spmd

D = 4096
FF = 11008
NCORES = 8
FSH = FF // NCORES            # 1376 rows of d_ff per core
NCH = (FSH + 127) // 128      # 11 f-chunks of <=128
LAST = FSH - 128 * (NCH - 1)  # 96 rows in the last chunk
NDC = D // 128                # 32 d-chunks
G = 4                         # d-chunks per gate DMA tile
NT = NDC // G                 # 8 gate tiles
GU = 8                        # d-chunks per up DMA tile (int8: same bytes)
NTU = NDC // GU               # 4 up tiles
ND2 = 5                       # paired down tiles (chunks 0..9)
HD = D // 2                   # 2048: half output width
F32 = mybir.dt.float32
F16 = mybir.dt.float16
I8 = mybir.dt.int8
ACT = mybir.ActivationFunctionType

# interleaved stream: gate finishes ~80% through so the x1 path overlaps
# the up tail; u3 follows the x1 path
ORDER = [
    ("g", 0), ("g", 1), ("u", 0), ("g", 2), ("u", 1), ("g", 3), ("g", 4),
    ("u", 2), ("g", 5), ("g", 6), ("g", 7),
]

_CACHE = {}


def _build_nc(thr_value):
    nc = bacc.Bacc("TRN2", target_bir_lowering=False, debug=False)

    x_d = nc.dram_tensor("x", [NDC, 128], F16, kind="ExternalInput")
    wg_d = nc.dram_tensor("wg", [NT, 128, G * FSH], F16, kind="ExternalInput")
    wu_d = nc.dram_tensor("wu", [NTU, 128, GU * FSH], I8, kind="ExternalInput")
    wd_d = nc.dram_tensor("wd", [ND2, 128, 2 * D], I8, kind="ExternalInput")
    wdL_d = nc.dram_tensor("wdL", [LAST, D], I8, kind="ExternalInput")
    sud_d = nc.dram_tensor("sud", [128, NCH], F32, kind="ExternalInput")
    out_d = nc.dram_tensor("out", [1, D], F32, kind="ExternalOutput")

    NSPL = ((0, 512), (512, 1024), (1024, FSH))

    with tile.TileContext(nc) as tc:
        with (
            tc.tile_pool(name="const", bufs=1) as const_pool,
            tc.tile_pool(name="wpool", bufs=3) as wpool,
            tc.tile_pool(name="u8pool", bufs=2) as u8pool,
            tc.tile_pool(name="u16pool", bufs=2) as u16pool,
            tc.tile_pool(name="d8pool", bufs=3) as d8pool,
            tc.tile_pool(name="d16pool", bufs=3) as d16pool,
            tc.tile_pool(name="acts", bufs=1) as acts,
        ):
            # x arrives [32,128]; transposing DMA (XBAR) lands it as
            # [128,32] chunk-major without a 128-descriptor broadcast storm.
            # On the sync ring, ahead of the weights: it is tiny and the
            # scalar ring's queue bring-up would otherwise delay it.
            x_sb = const_pool.tile([128, NDC], F16)
            nc.sync.dma_start(out=x_sb[:], in_=x_d.ap(), transpose=True)
            one_sb = const_pool.tile([1, 1], F16)
            nc.vector.memset(one_sb[:], 1.0)

            # warm the silu_and_others ACT table while the DMA stream runs
            warm = acts.tile([1, 1], F32)
            nc.vector.memset(warm[:], 1.0)
            nc.scalar.activation(warm[:], warm[:], ACT.Silu)
            nc.scalar.activation(warm[:], warm[:], ACT.Abs)

            x1row_sb = acts.tile([1, FSH], F16)
            urow_sb = acts.tile([1, FSH], F16)
            x1s = acts.tile([128, NCH], F32)
            absx = acts.tile([128, NCH], F32)
            mask = acts.tile([128, NCH], F32)
            ztmp = acts.tile([128, NCH], F32)
            zmA = acts.tile([128, NCH], F32)
            zm_sb = acts.tile([128, NCH], F16)
            sud_sb = acts.tile([128, NCH], F32)
            out_sb = acts.tile([1, D], F32)

            def cast_chunk(dst_ap, src_ap, on_act):
                if on_act:
                    nc.scalar.copy(dst_ap, src_ap)
                else:
                    nc.vector.tensor_scalar_mul(dst_ap, src_ap, 1.0)

            with tc.tile_pool(name="psA", bufs=1, space="PSUM") as psA:
                x1row = psA.tile([1, FSH], F32)
                urow = psA.tile([1, FSH], F32)
                x1tr = psA.tile([128, NCH], F32)
                utr = psA.tile([128, NCH], F32)
                nc.vector.memset(x1tr[:], 0.0)
                nc.vector.memset(utr[:], 0.0)

                def mm(accrow, c, rhs_ap, n0, n1):
                    nc.tensor.matmul(
                        out=accrow[0:1, n0:n1],
                        lhsT=x_sb[:, c : c + 1],
                        rhs=rhs_ap,
                        start=(c == 0),
                        stop=(c == NDC - 1),
                    )

                def gate_tile(t):
                    wt = wpool.tile([128, G * FSH], F16, tag="w", name="wt")
                    if t == 0:
                        # 4 chunk DMAs: the PE can start on the first chunk
                        # ~3us before a whole-tile transfer would land
                        for g in range(G):
                            cs = slice(g * FSH, (g + 1) * FSH)
                            nc.sync.dma_start(out=wt[:, cs], in_=wg_d.ap()[0][:, cs])
                    else:
                        nc.sync.dma_start(out=wt[:], in_=wg_d.ap()[t])
                    for g in range(G):
                        for n0, n1 in NSPL:
                            mm(x1row, G * t + g, wt[:, g * FSH + n0 : g * FSH + n1], n0, n1)

                def up_tile(t):
                    u8 = u8pool.tile([128, GU * FSH], I8, tag="u8", name="u8")
                    nc.sync.dma_start(out=u8[:], in_=wu_d.ap()[t])
                    uf = u16pool.tile([128, GU * FSH], F16, tag="uf", name="uf")
                    for g in range(GU):
                        cs = slice(g * FSH, (g + 1) * FSH)
                        cast_chunk(uf[:, cs], u8[:, cs], g % 8 in (2, 5, 7))
                        for n0, n1 in NSPL:
                            mm(urow, GU * t + g, uf[:, g * FSH + n0 : g * FSH + n1], n0, n1)

                def transpose_row(row_sb, dst):
                    # [1, FSH] row -> [128, NCH] partition-major via K=1 matmuls
                    for c in range(NCH):
                        pc = 128 if c < NCH - 1 else LAST
                        nc.tensor.matmul(
                            out=dst[:pc, c : c + 1],
                            lhsT=row_sb[0:1, c * 128 : c * 128 + pc],
                            rhs=one_sb[:],
                            start=True,
                            stop=True,
                        )

                for kind, t in ORDER:
                    (gate_tile if kind == "g" else up_tile)(t)
                # x1 post-processing overlaps the up tail
                nc.scalar.copy(x1row_sb[:], x1row[:])
                nc.scalar.dma_start(out=sud_sb[:], in_=sud_d.ap())
                transpose_row(x1row_sb, x1tr)
                nc.scalar.activation(x1s[:], x1tr[:], ACT.Silu)
                nc.scalar.activation(absx[:], x1s[:], ACT.Abs)
                nc.vector.tensor_scalar(
                    out=mask[:],
                    in0=absx[:],
                    scalar1=float(thr_value),
                    scalar2=None,
                    op0=mybir.AluOpType.is_gt,
                )
                up_tile(3)

                # split the PSUM->SBUF drain across Act+DVE: it gates the
                # down-projection start, so it must precede the down-tile
                # casts in both engines' queues
                HF = (FSH // 2) // 512 * 512  # 512-aligned split point
                nc.scalar.copy(urow_sb[0:1, :HF], urow[0:1, :HF])
                nc.vector.tensor_copy(urow_sb[0:1, HF:], urow[0:1, HF:])
                transpose_row(urow_sb, utr)
                nc.vector.tensor_mul(ztmp[:], utr[:], x1s[:])
                nc.vector.tensor_mul(zmA[:], ztmp[:], mask[:])
                nc.vector.tensor_mul(zm_sb[:], zmA[:], sud_sb[:])

                # prefetch + dequant the first down tiles so the DMA and
                # dequant engines don't idle across the pool transition
                dpre = []
                for t in range(3):
                    d8 = d8pool.tile([128, 2 * D], I8, tag="d8", name="d8")
                    nc.sync.dma_start(out=d8[:], in_=wd_d.ap()[t])
                    df = d16pool.tile([128, 2 * D], F16, tag="df", name="df")
                    for h in range(2):
                        hs = slice(h * D, (h + 1) * D)
                        cast_chunk(df[:, hs], d8[:, hs], (2 * t + h) % 2 == 1)
                    dpre.append(df)

            with tc.tile_pool(name="psB", bufs=1, space="PSUM") as psB:
                dn = psB.tile([1, D], F32)

                def down_mms(c, df_ap):
                    # df_ap: [pc, D] fp16 view of chunk c's dequantized rows
                    pc = 128 if c < NCH - 1 else LAST
                    for b in range(8):
                        nc.tensor.matmul(
                            out=dn[0:1, b * 512 : (b + 1) * 512],
                            lhsT=zm_sb[:pc, c : c + 1],
                            rhs=df_ap[:, b * 512 : (b + 1) * 512],
                            start=(c == 0),
                            stop=(c == NCH - 1),
                        )

                for t, df in enumerate(dpre):
                    down_mms(2 * t, df[:, :D])
                    down_mms(2 * t + 1, df[:, D:])
                for t in range(3, ND2):
                    d8 = d8pool.tile([128, 2 * D], I8, tag="d8", name="d8")
                    nc.sync.dma_start(out=d8[:], in_=wd_d.ap()[t])
                    df = d16pool.tile([128, 2 * D], F16, tag="df", name="df")
                    for h in range(2):
                        hs = slice(h * D, (h + 1) * D)
                        cast_chunk(df[:, hs], d8[:, hs], (2 * t + h) % 2 == 1)
                        down_mms(2 * t + h, df[:, hs])
                # last 96-row chunk
                d8L = d8pool.tile([128, 2 * D], I8, tag="d8", name="d8")
                nc.sync.dma_start(out=d8L[:LAST, :D], in_=wdL_d.ap())
                dfL = d16pool.tile([128, 2 * D], F16, tag="df", name="df")
                cast_chunk(dfL[:LAST, :D], d8L[:LAST, :D], False)
                down_mms(NCH - 1, dfL[:LAST, :D])
                # drain PSUM per bank as each accumulation closes; alternate
                # Act/DVE so the tail is half as long
                for b in range(8):
                    sl = slice(b * 512, (b + 1) * 512)
                    if b % 2 == 0:
                        nc.scalar.copy(out_sb[0:1, sl], dn[0:1, sl])
                    else:
                        nc.vector.tensor_copy(out_sb[0:1, sl], dn[0:1, sl])

            # two half stores so the first can fire while the tail drains
            nc.sync.dma_start(out=out_d.ap()[0:1, :HD], in_=out_sb[0:1, :HD])
            nc.sync.dma_start(out=out_d.ap()[0:1, HD:], in_=out_sb[0:1, HD:])

    nc.compile()
    return nc


def _get_nc(thr_value):
    key = ("nc", float(thr_value))
    if key not in _CACHE:
        _CACHE[key] = _build_nc(float(thr_value))
    return _CACHE[key]


def _quant_rows(w):
    """Per-row symmetric int8: returns (q int8, scale f32[rows])."""
    s = np.abs(w).max(axis=1) / 127.0
    s[s == 0] = 1.0
    q = np.clip(np.rint(w / s[:, None]), -127, 127).astype(np.int8)
    return q, s.astype(np.float32)


def make_in_maps(x, Wup, Wgatet, Wdownt):
    """Shard full inputs into the 8 per-core input maps."""
    x16 = np.asarray(x, dtype=np.float32).reshape(D).astype(np.float16)
    xb = np.ascontiguousarray(x16.reshape(NDC, 128))       # [32, 128]
    Wg16 = np.asarray(Wgatet, dtype=np.float32).astype(np.float16)  # [D, FF]
    Wup = np.asarray(Wup, dtype=np.float32)                # [FF, D]
    Wdownt = np.asarray(Wdownt, dtype=np.float32)          # [FF, D]
    in_maps = []
    for i in range(NCORES):
        sl = slice(i * FSH, (i + 1) * FSH)
        wg = (
            Wg16[:, sl]
            .reshape(NT, G, 128, FSH)
            .transpose(0, 2, 1, 3)
            .reshape(NT, 128, G * FSH)
        )
        wg = np.ascontiguousarray(wg)                      # [NT, 128, G*FSH]

        qu, su = _quant_rows(Wup[sl, :])                   # [FSH, D], [FSH]
        wu = (
            qu.T.reshape(NTU, GU, 128, FSH)
            .transpose(0, 2, 1, 3)
            .reshape(NTU, 128, GU * FSH)
        )
        wu = np.ascontiguousarray(wu)                      # [NTU, 128, GU*FSH]

        qd, sd = _quant_rows(Wdownt[sl, :])                # [FSH, D], [FSH]
        # paired down tiles: chunk 2t and 2t+1 side by side in the free dim
        wd = np.ascontiguousarray(
            qd[: ND2 * 256, :].reshape(ND2, 2, 128, D)
            .transpose(0, 2, 1, 3)
            .reshape(ND2, 128, 2 * D)
        )
        wdL = np.ascontiguousarray(qd[ND2 * 256 :, :])     # [96, D]

        sud = np.zeros(NCH * 128, dtype=np.float32)
        sud[:FSH] = su * sd
        sud = np.ascontiguousarray(sud.reshape(NCH, 128).T)  # [128, NCH]

        in_maps.append(
            {"x": xb, "wg": wg, "wu": wu, "wd": wd, "wdL": wdL, "sud": sud}
        )
    return in_maps


def run_sharded(x, Wup, Wgatet, Wdownt, threshold, trace=False, tmpdir=None):
    """Run on the 8 NeuronCores; returns (full_output, BassKernelResults)."""
    thr = float(np.asarray(threshold, dtype=np.float32).reshape(()))
    nc = _get_nc(thr)
    in_maps = make_in_maps(x, Wup, Wgatet, Wdownt)
    res = run_bass_kernel_spmd(
        nc, in_maps, list(range(NCORES)), trace=trace, tmpdir=tmpdir
    )
    # un-shard: sum the 8 partial down-projections
    acc = np.zeros(D, dtype=np.float64)
    for r in res.results:
        acc += r["out"].reshape(D).astype(np.float64)
    out = acc.astype(np.float32).reshape(1, 1, D)
    return out, res


def kernel(x, Wup, Wgatet, Wdownt, threshold):
    out, _ = run_sharded(x, Wup, Wgatet, Wdownt, threshold)
    return out


# revision 25
# speedup vs baseline: 1.1159x; 1.0146x over previous
# CATS-SwiGLU decode kernel for TRN2 (8 NeuronCores, SPMD tensor-parallel).
#
# Reference computation (decode path, B=S=1):
#   x1    = silu(x @ Wgatet)                  [1,1,dff]
#   flags = |x1| > threshold
#   z     = where(flags, (x @ Wup.T) * x1, 0) [1,1,dff]
#   out   = z @ Wdownt                        [1,1,d]
#
# Sharding: d_ff (11008) split across 8 cores (1376 rows each). Each core
# computes its z slice and a full-width partial down-projection; the host
# sums the 8 partials (the all-reduce of the TP hint, done on host).
#
# The kernel streams every weight byte exactly once; the design goals are
# fewer bytes and no idle engines:
#  - Wgatet streams as fp16 (it decides the CATS flags, keep it accurate);
#    Wup/Wdownt stream as int8 with per-row scales folded into the z vector
#    (exact, since z_f scales whole rows).
#  - int8 tiles are dequantized to fp16 one chunk at a time, alternating
#    between the DVE (tensor_scalar, 2x perf mode) and the otherwise-idle
#    Act engine (Copy) so neither paces the pipeline.
#  - All DMAs are uniform ~1.4-2MB tiles on the sync HWDGE ring (mixed
#    small tiles measurably drop the 16-engine stream rate); gate and up
#    tiles interleave so dequant overlaps the fp16 stream, and the first
#    gate tile is split into 4 chunk DMAs so the PE starts early.
#  - All GEMVs run on the TensorEngine as M=1 matmuls (x / z stationary,
#    weights moving) with **PE column tiling**: the three N-slices of each
#    chunk issue at tile_position col-groups 0/32/64, so up to three
#    matmuls stream concurrently through separate XBUSes (~2.4x PE
#    throughput). Their outputs land at PSUM partitions 0/32/64, which
#    also collapses the row accumulators to one bank each (partition-
#    sliced tiles share byte ranges) - the whole kernel fits one PSUM pool.
#  - Gate/up PSUM rows are transposed to [128,11] via K=1 matmuls against
#    a ones column so z is partition-major, ready as the down stationary.
#  - The threshold is baked into the mask op as an immediate (kernel cache
#    keyed on its value); x arrives [32,128] via a transposing (XBAR) DMA.
import sys

for _p in ("/opt/trn_rl_repo",):
    if _p not in sys.path:
        sys.path.insert(0, _p)

import numpy as np

import concourse.bass as bass
import concourse.tile as tile
from concourse import bacc, mybir
from concourse.bass_utils import run_bass_kernel_spmd

D = 4096
FF = 11008
NCORES = 8
FSH = FF // NCORES            # 1376 rows of d_ff per core
NCH = (FSH + 127) // 128      # 11 f-chunks of <=128
LAST = FSH - 128 * (NCH - 1)  # 96 rows in the last chunk
NDC = D // 128                # 32 d-chunks
G = 4                         # d-chunks per gate DMA tile
NT = NDC // G                 # 8 gate tiles
GU = 8                        # d-chunks per up DMA tile (int8: same bytes)
NTU = NDC // GU               # 4 up tiles
ND2 = 5                       # paired down tiles (chunks 0..9)
HD = D // 2                   # 2048: half output width
F32 = mybir.dt.float32
F16 = mybir.dt.float16
I8 = mybir.dt.int8
ACT = mybir.ActivationFunctionType

# gate/up N-slices -> (col-group, psum column range): three concurrent MMs
NSPL = ((0, 512, 0), (512, 1024, 32), (1024, FSH, 64))
# interleaved stream: gate finishes ~80% through so the x1 path overlaps
# the up tail; u3 follows the x1 path
ORDER = [
    ("g", 0), ("g", 1), ("u", 0), ("g", 2), ("u", 1), ("g", 3), ("g", 4),
    ("u", 2), ("g", 5), ("g", 6), ("g", 7),
]

_CACHE = {}


def _build_nc(thr_value):
    nc = bacc.Bacc("TRN2", target_bir_lowering=False, debug=False)

    x_d = nc.dram_tensor("x", [NDC, 128], F16, kind="ExternalInput")
    wg_d = nc.dram_tensor("wg", [NT, 128, G * FSH], F16, kind="ExternalInput")
    wu_d = nc.dram_tensor("wu", [NTU, 128, GU * FSH], I8, kind="ExternalInput")
    wd_d = nc.dram_tensor("wd", [ND2, 128, 2 * D], I8, kind="ExternalInput")
    wdL_d = nc.dram_tensor("wdL", [LAST, D], I8, kind="ExternalInput")
    sud_d = nc.dram_tensor("sud", [128, NCH], F32, kind="ExternalInput")
    out_d = nc.dram_tensor("out", [1, D], F32, kind="ExternalOutput")

    with tile.TileContext(nc) as tc:
        with (
            tc.tile_pool(name="const", bufs=1) as const_pool,
            tc.tile_pool(name="wpool", bufs=3) as wpool,
            tc.tile_pool(name="u8pool", bufs=2) as u8pool,
            tc.tile_pool(name="u16pool", bufs=2) as u16pool,
            tc.tile_pool(name="d8pool", bufs=3) as d8pool,
            tc.tile_pool(name="d16pool", bufs=3) as d16pool,
            tc.tile_pool(name="acts", bufs=1) as acts,
            tc.tile_pool(name="psum", bufs=1, space="PSUM") as psum,
        ):
            # x arrives [32,128]; transposing DMA (XBAR) lands it as
            # [128,32] chunk-major, on the sync ring ahead of the weights
            x_sb = const_pool.tile([128, NDC], F16)
            nc.sync.dma_start(out=x_sb[:], in_=x_d.ap(), transpose=True)
            ones = const_pool.tile([128, 1], F16)
            nc.vector.memset(ones[:], 1.0)

            # warm the silu_and_others ACT table while the DMA stream runs
            warm = acts.tile([1, 1], F32)
            nc.vector.memset(warm[:], 1.0)
            nc.scalar.activation(warm[:], warm[:], ACT.Silu)
            nc.scalar.activation(warm[:], warm[:], ACT.Abs)

            # PSUM: partition-sliced accumulators (p0/p32/p64 share banks)
            x1row = psum.tile([128, 512], F32)   # [si] rows: f si*512..
            urow = psum.tile([128, 512], F32)
            x1tr = psum.tile([128, NCH], F32)
            utr = psum.tile([128, NCH], F32)
            dn = psum.tile([128, 3 * 512], F32)  # p(32*(b%3)), col (b//3)*512
            nc.vector.memset(x1tr[:], 0.0)
            nc.vector.memset(utr[:], 0.0)

            x1row_sb = acts.tile([128, 512], F16)
            urow_sb = acts.tile([128, 512], F16)
            x1s = acts.tile([128, NCH], F32)
            absx = acts.tile([128, NCH], F32)
            mask = acts.tile([128, NCH], F32)
            ztmp = acts.tile([128, NCH], F32)
            zmA = acts.tile([128, NCH], F32)
            zm_sb = acts.tile([128, NCH], F16)
            sud_sb = acts.tile([128, NCH], F32)
            out_sb = acts.tile([128, 3 * 512], F32)

            def cast_chunk(dst_ap, src_ap, on_act):
                if on_act:
                    nc.scalar.copy(dst_ap, src_ap)
                else:
                    nc.vector.tensor_scalar_mul(dst_ap, src_ap, 1.0)

            def mm(accrow, c, rhs_ap, n0, si):
                # col-group si: out at partition 32*si, cols n0-relative
                nc.tensor.matmul(
                    out=accrow[32 * si : 32 * si + 1, 0 : rhs_ap.shape[-1]],
                    lhsT=x_sb[:, c : c + 1],
                    rhs=rhs_ap,
                    start=(c == 0),
                    stop=(c == NDC - 1),
                )

            def gate_tile(t):
                wt = wpool.tile([128, G * FSH], F16, tag="w", name="wt")
                if t == 0:
                    # 4 chunk DMAs: the PE can start on the first chunk
                    # ~3us before a whole-tile transfer would land
                    for g in range(G):
                        cs = slice(g * FSH, (g + 1) * FSH)
                        nc.sync.dma_start(out=wt[:, cs], in_=wg_d.ap()[0][:, cs])
                else:
                    nc.sync.dma_start(out=wt[:], in_=wg_d.ap()[t])
                for g in range(G):
                    for n0, n1, si in NSPL:
                        mm(x1row, G * t + g, wt[:, g * FSH + n0 : g * FSH + n1], n0, si // 32)

            def up_tile(t):
                u8 = u8pool.tile([128, GU * FSH], I8, tag="u8", name="u8")
                nc.sync.dma_start(out=u8[:], in_=wu_d.ap()[t])
                uf = u16pool.tile([128, GU * FSH], F16, tag="uf", name="uf")
                for g in range(GU):
                    cs = slice(g * FSH, (g + 1) * FSH)
                    cast_chunk(uf[:, cs], u8[:, cs], g % 8 in (2, 5, 7))
                    for n0, n1, si in NSPL:
                        mm(urow, GU * t + g, uf[:, g * FSH + n0 : g * FSH + n1], n0, si // 32)

            def row_pieces(tile_):
                # (partition, col0, cols) per 512-wide third of the row
                return ((0, 0, 512), (32, 512, 512), (64, 1024, FSH - 1024))

            def drain_row(row_ps, row_sb):
                # PSUM->SBUF f16, one piece per engine flavor
                for i, (p, f0, w) in enumerate(row_pieces(None)):
                    src = row_ps[p : p + 1, 0:w]
                    dst = row_sb[p : p + 1, 0:w]
                    if i % 2 == 0:
                        nc.scalar.copy(dst, src)
                    else:
                        nc.vector.tensor_copy(dst, src)

            def transpose_row(row_sb, dst):
                # [128-sliced row] -> [128, NCH] partition-major via K=1
                # matmuls; lhsT/rhs partition base follows the row piece
                for c in range(NCH):
                    pc = 128 if c < NCH - 1 else LAST
                    p = 32 * ((c * 128) // 512)
                    f0 = c * 128 - (p // 32) * 512
                    nc.tensor.matmul(
                        out=dst[:pc, c : c + 1],
                        lhsT=row_sb[p : p + 1, f0 : f0 + pc],
                        rhs=ones[p : p + 1, :],
                        start=True,
                        stop=True,
                    )

            for kind, t in ORDER:
                (gate_tile if kind == "g" else up_tile)(t)
            # x1 post-processing overlaps the up tail
            drain_row(x1row, x1row_sb)
            nc.scalar.dma_start(out=sud_sb[:], in_=sud_d.ap())
            transpose_row(x1row_sb, x1tr)
            nc.scalar.activation(x1s[:], x1tr[:], ACT.Silu)
            nc.scalar.activation(absx[:], x1s[:], ACT.Abs)
            nc.vector.tensor_scalar(
                out=mask[:],
                in0=absx[:],
                scalar1=float(thr_value),
                scalar2=None,
                op0=mybir.AluOpType.is_gt,
            )
            up_tile(3)

            # z-chain: must precede the down-tile casts in the engine queues
            drain_row(urow, urow_sb)
            transpose_row(urow_sb, utr)
            nc.vector.tensor_mul(ztmp[:], utr[:], x1s[:])
            nc.vector.tensor_mul(zmA[:], ztmp[:], mask[:])
            nc.vector.tensor_mul(zm_sb[:], zmA[:], sud_sb[:])

            def down_mms(c, df_ap):
                # df_ap: [pc, D] fp16 view of chunk c's dequantized rows;
                # 8 N-slices issue round-robin over col-groups 0/32/64
                pc = 128 if c < NCH - 1 else LAST
                for b in range(8):
                    p = 32 * (b % 3)
                    col = (b // 3) * 512
                    nc.tensor.matmul(
                        out=dn[p : p + 1, col : col + 512],
                        lhsT=zm_sb[:pc, c : c + 1],
                        rhs=df_ap[:, b * 512 : (b + 1) * 512],
                        start=(c == 0),
                        stop=(c == NCH - 1),
                    )

            dpre = []
            for t in range(3):
                d8 = d8pool.tile([128, 2 * D], I8, tag="d8", name="d8")
                nc.sync.dma_start(out=d8[:], in_=wd_d.ap()[t])
                df = d16pool.tile([128, 2 * D], F16, tag="df", name="df")
                for h in range(2):
                    hs = slice(h * D, (h + 1) * D)
                    cast_chunk(df[:, hs], d8[:, hs], (2 * t + h) % 2 == 1)
                dpre.append(df)

            for t, df in enumerate(dpre):
                down_mms(2 * t, df[:, :D])
                down_mms(2 * t + 1, df[:, D:])
            for t in range(3, ND2):
                d8 = d8pool.tile([128, 2 * D], I8, tag="d8", name="d8")
                nc.sync.dma_start(out=d8[:], in_=wd_d.ap()[t])
                df = d16pool.tile([128, 2 * D], F16, tag="df", name="df")
                for h in range(2):
                    hs = slice(h * D, (h + 1) * D)
                    cast_chunk(df[:, hs], d8[:, hs], (2 * t + h) % 2 == 1)
                    down_mms(2 * t + h, df[:, hs])
            # last 96-row chunk
            d8L = d8pool.tile([128, 2 * D], I8, tag="d8", name="d8")
            nc.sync.dma_start(out=d8L[:LAST, :D], in_=wdL_d.ap())
            dfL = d16pool.tile([128, 2 * D], F16, tag="df", name="df")
            cast_chunk(dfL[:LAST, :D], d8L[:LAST, :D], False)
            down_mms(NCH - 1, dfL[:LAST, :D])

            # drain dn per 512-slice as each accumulation closes
            for b in range(8):
                p = 32 * (b % 3)
                col = (b // 3) * 512
                src = dn[p : p + 1, col : col + 512]
                dst = out_sb[p : p + 1, col : col + 512]
                if b % 2 == 0:
                    nc.scalar.copy(dst, src)
                else:
                    nc.vector.tensor_copy(dst, src)

            # three strided stores: partition row p holds out[d] for
            # d = (3*j + p/32)*512 .. +512, j = 0..2 (j<2 for p64)
            for p, nblk in ((0, 3), (32, 3), (64, 2)):
                dst = bass.AP(
                    tensor=out_d.ap().tensor,
                    offset=(p // 32) * 512,
                    ap=[[3 * 512, nblk], [1, 512]],
                )
                nc.sync.dma_start(out=dst, in_=out_sb[p : p + 1, 0 : nblk * 512])

    nc.compile()
    return nc


def _get_nc(thr_value):
    key = ("nc", float(thr_value))
    if key not in _CACHE:
        _CACHE[key] = _build_nc(float(thr_value))
    return _CACHE[key]


def _quant_rows(w):
    """Per-row symmetric int8: returns (q int8, scale f32[rows])."""
    s = np.abs(w).max(axis=1) / 127.0
    s[s == 0] = 1.0
    q = np.clip(np.rint(w / s[:, None]), -127, 127).astype(np.int8)
    return q, s.astype(np.float32)


def make_in_maps(x, Wup, Wgatet, Wdownt):
    """Shard full inputs into the 8 per-core input maps."""
    x16 = np.asarray(x, dtype=np.float32).reshape(D).astype(np.float16)
    xb = np.ascontiguousarray(x16.reshape(NDC, 128))       # [32, 128]
    Wg16 = np.asarray(Wgatet, dtype=np.float32).astype(np.float16)  # [D, FF]
    Wup = np.asarray(Wup, dtype=np.float32)                # [FF, D]
    Wdownt = np.asarray(Wdownt, dtype=np.float32)          # [FF, D]
    in_maps = []
    for i in range(NCORES):
        sl = slice(i * FSH, (i + 1) * FSH)
        wg = (
            Wg16[:, sl]
            .reshape(NT, G, 128, FSH)
            .transpose(0, 2, 1, 3)
            .reshape(NT, 128, G * FSH)
        )
        wg = np.ascontiguousarray(wg)                      # [NT, 128, G*FSH]

        qu, su = _quant_rows(Wup[sl, :])                   # [FSH, D], [FSH]
        wu = (
            qu.T.reshape(NTU, GU, 128, FSH)
            .transpose(0, 2, 1, 3)
            .reshape(NTU, 128, GU * FSH)
        )
        wu = np.ascontiguousarray(wu)                      # [NTU, 128, GU*FSH]

        qd, sd = _quant_rows(Wdownt[sl, :])                # [FSH, D], [FSH]
        # paired down tiles: chunk 2t and 2t+1 side by side in the free dim
        wd = np.ascontiguousarray(
            qd[: ND2 * 256, :].reshape(ND2, 2, 128, D)
            .transpose(0, 2, 1, 3)
            .reshape(ND2, 128, 2 * D)
        )
        wdL = np.ascontiguousarray(qd[ND2 * 256 :, :])     # [96, D]

        sud = np.zeros(NCH * 128, dtype=np.float32)
        sud[:FSH] = su * sd
        sud = np.ascontiguousarray(sud.reshape(NCH, 128).T)  # [128, NCH]

        in_maps.append(
            {"x": xb, "wg": wg, "wu": wu, "wd": wd, "wdL": wdL, "sud": sud}
        )
    return in_maps


def run_sharded(x, Wup, Wgatet, Wdownt, threshold, trace=False, tmpdir=None):
    """Run on the 8 NeuronCores; returns (full_output, BassKernelResults)."""
    thr = float(np.asarray(threshold, dtype=np.float32).reshape(()))
    nc = _get_nc(thr)
    in_maps = make_in_maps(x, Wup, Wgatet, Wdownt)
    res = run_bass_kernel_spmd(
        nc, in_maps, list(range(NCORES)), trace=trace, tmpdir=tmpdir
    )
    # un-shard: sum the 8 partial down-projections
    acc = np.zeros(D, dtype=np.float64)
    for r in res.results:
        acc += r["out"].reshape(D).astype(np.float64)
    out = acc.astype(np.float32).reshape(1, 1, D)
    return out, res


def kernel(x, Wup, Wgatet, Wdownt, threshold):
    out, _ = run_sharded(x, Wup, Wgatet, Wdownt, threshold)
    return out


# revision 28
# speedup vs baseline: 1.1802x; 1.0576x over previous
# CATS-SwiGLU decode kernel for TRN2 (8 NeuronCores, SPMD tensor-parallel).
#
# Reference computation (decode path, B=S=1):
#   x1    = silu(x @ Wgatet)                  [1,1,dff]
#   flags = |x1| > threshold
#   z     = where(flags, (x @ Wup.T) * x1, 0) [1,1,dff]
#   out   = z @ Wdownt                        [1,1,d]
#
# Sharding: d_ff (11008) split across 8 cores (1376 rows each). Each core
# computes its z slice and a full-width partial down-projection; the host
# sums the 8 partials (the all-reduce of the TP hint, done on host).
#
# The kernel streams every weight byte exactly once; the design goals are
# fewer bytes and no idle engines:
#  - Wgatet streams as fp16 (it decides the CATS flags, keep it accurate);
#    Wup/Wdownt stream as int8 with per-row scales folded into the z vector
#    (exact, since z_f scales whole rows).
#  - int8 tiles are dequantized to fp16 one chunk at a time, alternating
#    between the DVE (tensor_scalar, 2x perf mode) and the otherwise-idle
#    Act engine (Copy) so neither paces the pipeline.
#  - All DMAs are uniform ~1.4-2MB tiles on the sync HWDGE ring (mixed
#    small tiles measurably drop the 16-engine stream rate); gate and up
#    tiles interleave so dequant overlaps the fp16 stream, and the first
#    gate tile is split into 4 chunk DMAs so the PE starts early.
#  - All GEMVs run on the TensorEngine as M=1 matmuls (x / z stationary,
#    weights moving) with **PE column tiling**: the three N-slices of each
#    chunk issue at tile_position col-groups 0/32/64, so up to three
#    matmuls stream concurrently through separate XBUSes (~2.4x PE
#    throughput). Their outputs land at PSUM partitions 0/32/64, which
#    also collapses the row accumulators to one bank each (partition-
#    sliced tiles share byte ranges) - the whole kernel fits one PSUM pool.
#  - Gate/up PSUM rows are transposed to [128,11] via K=1 matmuls against
#    a ones column so z is partition-major, ready as the down stationary.
#  - The threshold is baked into the mask op as an immediate (kernel cache
#    keyed on its value); x arrives [32,128] via a transposing (XBAR) DMA.
import sys

for _p in ("/opt/trn_rl_repo",):
    if _p not in sys.path:
        sys.path.insert(0, _p)

import numpy as np

import concourse.bass as bass
import concourse.tile as tile
from concourse import bacc, mybir
from concourse.bass_utils import run_bass_kernel_spmd

D = 4096
FF = 11008
NCORES = 8
FSH = FF // NCORES            # 1376 rows of d_ff per core
NCH = (FSH + 127) // 128      # 11 f-chunks of <=128
LAST = FSH - 128 * (NCH - 1)  # 96 rows in the last chunk
NDC = D // 128                # 32 d-chunks
G = 4                         # d-chunks per gate DMA tile
NT = NDC // G                 # 8 gate tiles
GU = 8                        # d-chunks per up DMA tile (int8: same bytes)
NTU = NDC // GU               # 4 up tiles
ND2 = 5                       # paired down tiles (chunks 0..9)
HD = D // 2                   # 2048: half output width
F32 = mybir.dt.float32
F16 = mybir.dt.float16
I8 = mybir.dt.int8
ACT = mybir.ActivationFunctionType

# gate/up N-slices -> (col-group, psum column range): three concurrent MMs
NSPL = ((0, 512, 0), (512, 1024, 32), (1024, FSH, 64))
# interleaved stream: gate finishes ~80% through so the x1 path overlaps
# the up tail; u3 follows the x1 path
ORDER = [
    ("g", 0), ("g", 1), ("u", 0), ("g", 2), ("u", 1), ("g", 3), ("g", 4),
    ("u", 2), ("g", 5), ("g", 6), ("g", 7),
]

_CACHE = {}


def _build_nc(thr_value):
    nc = bacc.Bacc("TRN2", target_bir_lowering=False, debug=False)

    x_d = nc.dram_tensor("x", [NDC, 128], F16, kind="ExternalInput")
    wg_d = nc.dram_tensor("wg", [NT, 128, G * FSH], F16, kind="ExternalInput")
    wu_d = nc.dram_tensor("wu", [NTU, 128, GU * FSH], I8, kind="ExternalInput")
    wd_d = nc.dram_tensor("wd", [ND2, 128, 2 * D], I8, kind="ExternalInput")
    wdL_d = nc.dram_tensor("wdL", [LAST, D], I8, kind="ExternalInput")
    sud_d = nc.dram_tensor("sud", [128, NCH], F32, kind="ExternalInput")
    out_d = nc.dram_tensor("out", [1, D], F32, kind="ExternalOutput")

    with tile.TileContext(nc) as tc:
        with (
            tc.tile_pool(name="const", bufs=1) as const_pool,
            tc.tile_pool(name="wpool", bufs=3) as wpool,
            tc.tile_pool(name="u8pool", bufs=2) as u8pool,
            tc.tile_pool(name="u16pool", bufs=2) as u16pool,
            tc.tile_pool(name="d8pool", bufs=6) as d8pool,
            tc.tile_pool(name="d16pool", bufs=3) as d16pool,
            tc.tile_pool(name="acts", bufs=1) as acts,
            tc.tile_pool(name="psum", bufs=1, space="PSUM") as psum,
        ):
            # x arrives [32,128]; transposing DMA (XBAR) lands it as
            # [128,32] chunk-major. Scalar ring: the XBAR path is slow for
            # small transfers and would stall the weight stream on sync.
            x_sb = const_pool.tile([128, NDC], F16)
            nc.scalar.dma_start(out=x_sb[:], in_=x_d.ap(), transpose=True)
            ones = const_pool.tile([128, 1], F16)
            nc.vector.memset(ones[:], 1.0)

            # warm the silu_and_others ACT table while the DMA stream runs
            warm = acts.tile([1, 1], F32)
            nc.vector.memset(warm[:], 1.0)
            nc.scalar.activation(warm[:], warm[:], ACT.Silu)
            nc.scalar.activation(warm[:], warm[:], ACT.Abs)

            # PSUM: partition-sliced accumulators (p0/p32/p64 share banks)
            x1row = psum.tile([128, 512], F32)   # [si] rows: f si*512..
            urow = psum.tile([128, 512], F32)
            x1tr = psum.tile([128, NCH], F32)
            utr = psum.tile([128, NCH], F32)
            dn = psum.tile([128, 3 * 512], F32)  # p(32*(b%3)), col (b//3)*512
            nc.vector.memset(x1tr[:], 0.0)
            nc.vector.memset(utr[:], 0.0)

            x1row_sb = acts.tile([128, 512], F16)
            urow_sb = acts.tile([128, 512], F16)
            x1s = acts.tile([128, NCH], F32)
            absx = acts.tile([128, NCH], F32)
            mask = acts.tile([128, NCH], F32)
            ztmp = acts.tile([128, NCH], F32)
            zmA = acts.tile([128, NCH], F32)
            zm_sb = acts.tile([128, NCH], F16)
            sud_sb = acts.tile([128, NCH], F32)
            out_sb = acts.tile([128, 3 * 512], F32)

            def cast_chunk(dst_ap, src_ap, on_act):
                if on_act:
                    nc.scalar.copy(dst_ap, src_ap)
                else:
                    nc.vector.tensor_scalar_mul(dst_ap, src_ap, 1.0)

            def mm(accrow, c, rhs_ap, n0, si):
                # col-group si: out at partition 32*si, cols n0-relative
                nc.tensor.matmul(
                    out=accrow[32 * si : 32 * si + 1, 0 : rhs_ap.shape[-1]],
                    lhsT=x_sb[:, c : c + 1],
                    rhs=rhs_ap,
                    start=(c == 0),
                    stop=(c == NDC - 1),
                )

            def gate_tile(t):
                wt = wpool.tile([128, G * FSH], F16, tag="w", name="wt")
                if t == 0:
                    # 4 chunk DMAs: the PE can start on the first chunk
                    # ~3us before a whole-tile transfer would land
                    for g in range(G):
                        cs = slice(g * FSH, (g + 1) * FSH)
                        nc.sync.dma_start(out=wt[:, cs], in_=wg_d.ap()[0][:, cs])
                else:
                    nc.sync.dma_start(out=wt[:], in_=wg_d.ap()[t])
                for g in range(G):
                    for n0, n1, si in NSPL:
                        mm(x1row, G * t + g, wt[:, g * FSH + n0 : g * FSH + n1], n0, si // 32)

            def up_tile(t):
                u8 = u8pool.tile([128, GU * FSH], I8, tag="u8", name="u8")
                nc.sync.dma_start(out=u8[:], in_=wu_d.ap()[t])
                uf = u16pool.tile([128, GU * FSH], F16, tag="uf", name="uf")
                for g in range(GU):
                    cs = slice(g * FSH, (g + 1) * FSH)
                    cast_chunk(uf[:, cs], u8[:, cs], g % 8 in (2, 5, 7))
                    for n0, n1, si in NSPL:
                        mm(urow, GU * t + g, uf[:, g * FSH + n0 : g * FSH + n1], n0, si // 32)

            def row_pieces(tile_):
                # (partition, col0, cols) per 512-wide third of the row
                return ((0, 0, 512), (32, 512, 512), (64, 1024, FSH - 1024))

            def drain_row(row_ps, row_sb):
                # PSUM->SBUF f16, one piece per engine flavor
                for i, (p, f0, w) in enumerate(row_pieces(None)):
                    src = row_ps[p : p + 1, 0:w]
                    dst = row_sb[p : p + 1, 0:w]
                    if i % 2 == 0:
                        nc.scalar.copy(dst, src)
                    else:
                        nc.vector.tensor_copy(dst, src)

            def transpose_row(row_sb, dst):
                # [128-sliced row] -> [128, NCH] partition-major via K=1
                # matmuls; lhsT/rhs partition base follows the row piece
                for c in range(NCH):
                    pc = 128 if c < NCH - 1 else LAST
                    p = 32 * ((c * 128) // 512)
                    f0 = c * 128 - (p // 32) * 512
                    nc.tensor.matmul(
                        out=dst[:pc, c : c + 1],
                        lhsT=row_sb[p : p + 1, f0 : f0 + pc],
                        rhs=ones[p : p + 1, :],
                        start=True,
                        stop=True,
                    )

            for kind, t in ORDER:
                (gate_tile if kind == "g" else up_tile)(t)
            # x1 post-processing overlaps the up tail
            drain_row(x1row, x1row_sb)
            nc.scalar.dma_start(out=sud_sb[:], in_=sud_d.ap())
            transpose_row(x1row_sb, x1tr)
            nc.scalar.activation(x1s[:], x1tr[:], ACT.Silu)
            nc.scalar.activation(absx[:], x1s[:], ACT.Abs)
            nc.vector.tensor_scalar(
                out=mask[:],
                in0=absx[:],
                scalar1=float(thr_value),
                scalar2=None,
                op0=mybir.AluOpType.is_gt,
            )
            up_tile(3)

            # z-chain: must precede the down-tile casts in the engine queues
            drain_row(urow, urow_sb)
            transpose_row(urow_sb, utr)
            nc.vector.tensor_mul(ztmp[:], utr[:], x1s[:])
            nc.vector.tensor_mul(zmA[:], ztmp[:], mask[:])
            nc.vector.tensor_mul(zm_sb[:], zmA[:], sud_sb[:])

            def down_mms(c, df_ap):
                # df_ap: [pc, D] fp16 view of chunk c's dequantized rows;
                # 8 N-slices issue round-robin over col-groups 0/32/64
                pc = 128 if c < NCH - 1 else LAST
                for b in range(8):
                    p = 32 * (b % 3)
                    col = (b // 3) * 512
                    nc.tensor.matmul(
                        out=dn[p : p + 1, col : col + 512],
                        lhsT=zm_sb[:pc, c : c + 1],
                        rhs=df_ap[:, b * 512 : (b + 1) * 512],
                        start=(c == 0),
                        stop=(c == NCH - 1),
                    )

            # down dequant: split each 4096-col chunk between DVE and Act
            # (rate-balanced at 2560) so the per-chunk cast latency is
            # ~1.4us with both engines working every chunk
            DSP = 2560

            def cast_down(df_ap, d8_ap, p=128):
                nc.vector.tensor_scalar_mul(df_ap[:p, :DSP], d8_ap[:p, :DSP], 1.0)
                nc.scalar.copy(df_ap[:p, DSP:], d8_ap[:p, DSP:])

            for t in range(ND2):
                d8 = d8pool.tile([128, 2 * D], I8, tag="d8", name="d8")
                nc.sync.dma_start(out=d8[:], in_=wd_d.ap()[t])
                df = d16pool.tile([128, 2 * D], F16, tag="df", name="df")
                for h in range(2):
                    hs = slice(h * D, (h + 1) * D)
                    cast_down(df[:, hs], d8[:, hs])
                    down_mms(2 * t + h, df[:, hs])
            # last 96-row chunk
            d8L = d8pool.tile([128, 2 * D], I8, tag="d8", name="d8")
            nc.sync.dma_start(out=d8L[:LAST, :D], in_=wdL_d.ap())
            dfL = d16pool.tile([128, 2 * D], F16, tag="df", name="df")
            cast_down(dfL[:, :D], d8L[:, :D], p=LAST)
            down_mms(NCH - 1, dfL[:LAST, :D])

            # drain dn per 512-slice as each accumulation closes
            for b in range(8):
                p = 32 * (b % 3)
                col = (b // 3) * 512
                src = dn[p : p + 1, col : col + 512]
                dst = out_sb[p : p + 1, col : col + 512]
                if b % 2 == 0:
                    nc.scalar.copy(dst, src)
                else:
                    nc.vector.tensor_copy(dst, src)

            # three strided stores: partition row p holds out[d] for
            # d = (3*j + p/32)*512 .. +512, j = 0..2 (j<2 for p64)
            for p, nblk in ((0, 3), (32, 3), (64, 2)):
                dst = bass.AP(
                    tensor=out_d.ap().tensor,
                    offset=(p // 32) * 512,
                    ap=[[3 * 512, nblk], [1, 512]],
                )
                nc.sync.dma_start(out=dst, in_=out_sb[p : p + 1, 0 : nblk * 512])

    nc.compile()
    return nc


def _get_nc(thr_value):
    key = ("nc", float(thr_value))
    if key not in _CACHE:
        _CACHE[key] = _build_nc(float(thr_value))
    return _CACHE[key]


def _quant_rows(w):
    """Per-row symmetric int8: returns (q int8, scale f32[rows])."""
    s = np.abs(w).max(axis=1) / 127.0
    s[s == 0] = 1.0
    q = np.clip(np.rint(w / s[:, None]), -127, 127).astype(np.int8)
    return q, s.astype(np.float32)


def make_in_maps(x, Wup, Wgatet, Wdownt):
    """Shard full inputs into the 8 per-core input maps."""
    x16 = np.asarray(x, dtype=np.float32).reshape(D).astype(np.float16)
    xb = np.ascontiguousarray(x16.reshape(NDC, 128))       # [32, 128]
    Wg16 = np.asarray(Wgatet, dtype=np.float32).astype(np.float16)  # [D, FF]
    Wup = np.asarray(Wup, dtype=np.float32)                # [FF, D]
    Wdownt = np.asarray(Wdownt, dtype=np.float32)          # [FF, D]
    in_maps = []
    for i in range(NCORES):
        sl = slice(i * FSH, (i + 1) * FSH)
        wg = (
            Wg16[:, sl]
            .reshape(NT, G, 128, FSH)
            .transpose(0, 2, 1, 3)
            .reshape(NT, 128, G * FSH)
        )
        wg = np.ascontiguousarray(wg)                      # [NT, 128, G*FSH]

        qu, su = _quant_rows(Wup[sl, :])                   # [FSH, D], [FSH]
        wu = (
            qu.T.reshape(NTU, GU, 128, FSH)
            .transpose(0, 2, 1, 3)
            .reshape(NTU, 128, GU * FSH)
        )
        wu = np.ascontiguousarray(wu)                      # [NTU, 128, GU*FSH]

        qd, sd = _quant_rows(Wdownt[sl, :])                # [FSH, D], [FSH]
        # paired down tiles: chunk 2t and 2t+1 side by side in the free dim
        wd = np.ascontiguousarray(
            qd[: ND2 * 256, :].reshape(ND2, 2, 128, D)
            .transpose(0, 2, 1, 3)
            .reshape(ND2, 128, 2 * D)
        )
        wdL = np.ascontiguousarray(qd[ND2 * 256 :, :])     # [96, D]

        sud = np.zeros(NCH * 128, dtype=np.float32)
        sud[:FSH] = su * sd
        sud = np.ascontiguousarray(sud.reshape(NCH, 128).T)  # [128, NCH]

        in_maps.append(
            {"x": xb, "wg": wg, "wu": wu, "wd": wd, "wdL": wdL, "sud": sud}
        )
    return in_maps


def run_sharded(x, Wup, Wgatet, Wdownt, threshold, trace=False, tmpdir=None):
    """Run on the 8 NeuronCores; returns (full_output, BassKernelResults)."""
    thr = float(np.asarray(threshold, dtype=np.float32).reshape(()))
    nc = _get_nc(thr)
    in_maps = make_in_maps(x, Wup, Wgatet, Wdownt)
    res = run_bass_kernel_spmd(
        nc, in_maps, list(range(NCORES)), trace=trace, tmpdir=tmpdir
    )
    # un-shard: sum the 8 partial down-projections
    acc = np.zeros(D, dtype=np.float64)
    for r in res.results:
        acc += r["out"].reshape(D).astype(np.float64)
    out = acc.astype(np.float32).reshape(1, 1, D)
    return out, res


def kernel(x, Wup, Wgatet, Wdownt, threshold):
    out, _ = run_sharded(x, Wup, Wgatet, Wdownt, threshold)
    return out
